# revision 1
# baseline (speedup 1.0000x reference)
"""Trainium2 Bass kernel for GCNConv(1->1) + per-graph FC over 100k disjoint
50-node graphs (5M nodes, 80M random edges).

Sharding strategy (host side = index plumbing / data layout only, no float
arithmetic): edges are sharded across the 8 cores by target-node range
(data-parallel over graphs, per the sharding hint), sorted by target column
into fixed-length per-partition streams, and each edge is co-located with its
source feature x[row] and source in-degree (uint8).  All floating-point math
(degree normalization rsqrt, message values, segment sums, conv bias, FC)
runs on the NeuronCores:

  per round r (32 rounds/core), stream tile [128 partitions, L]:
    ACT : s = sqrt(deg_src + 1)            (uint8 -> f32)
    DVE : d = 1/s ; v = xr * d ; S = cumsum(v)   (tensor_tensor_scan)
    GPS : X[p, c] = S_p[endpos[c]]         (indirect_copy, shared idx/group)
    PE  : B = group-prefix(S totals)       (block-triangular matmul)
    DVE : M = (X + B) * ownermask
    PE  : V[g, c] = sum over group partitions of M   (group-ones matmul)
  then per-column:  partial = V - V_shifted ; dis = rsqrt(1+deg)
  h = w*dis*(partial + dis*x) + b ;  logits = per-graph h @ fc_w.T + fc_b
"""
import numpy as np

# ---------------- constants (hardcoded shapes/sharding) ----------------
N = 5_000_000
E = 80_000_000
NCORES = 8
COLS_CORE = 625_000          # real cols per core
PCOLS = 627_200              # padded cols per core = 128*4900
NG = 8                       # partition groups (of 16) per core
NR = 32                      # rounds per core
CG = 2450                    # cols per (group, round)
L = 2816                     # stream slots per partition per round
NI = 2464                    # idx slots per (group, round) = 1 + CG + 13 pad
NIW = NI // 16               # wrapped idx columns = 154
NSTREAM = NCORES * NR * 128  # 32768 partition-streams
GRAPHS_CORE = 12_500
PGRAPHS = PCOLS // 50        # 12544 incl pad graphs

# col->p16 assignment inside a (group, round): 2 blocks of 154 + 14 of 153
_BLK = [154, 154] + [153] * 14
_BOUND = np.concatenate([[0], np.cumsum(_BLK)])   # [17], last = 2450
assert _BOUND[-1] == CG
PMAP = np.repeat(np.arange(16), _BLK).astype(np.int64)   # [2450] -> p16

_PROGRAM = {}


def _host_prep(x, edge_index, conv_w, conv_b, fc_w, fc_b):
    x = np.asarray(x, dtype=np.float32).reshape(-1)
    row = np.asarray(edge_index[0])
    col = np.asarray(edge_index[1])

    deg = np.bincount(col, minlength=N)          # in-degree (no self loop)
    assert deg.max() <= 255, deg.max()
    deg8 = deg.astype(np.uint8)

    # edge -> (core, g, r, idx, p16) and stream id
    core = col // COLS_CORE
    local = col - core * COLS_CORE
    g = local // 78_400
    rem = local - g * 78_400
    r = rem // CG
    idx = rem - r * CG
    p16 = PMAP[idx]
    stream = ((core * NR + r) * NG + g) * 16 + p16          # [E] < 32768
    key = (stream * 2560 + idx).astype(np.int32)
    order = np.argsort(key, kind="stable")
    del key, core, local, g, rem, r, idx, p16

    stream_s = stream[order]
    scount = np.bincount(stream_s, minlength=NSTREAM)
    assert scount.max() <= L - 1, scount.max()
    sstart = np.concatenate([[0], np.cumsum(scount)[:-1]])
    slot = (np.arange(E, dtype=np.int64) - sstart[stream_s]) + 1  # slot0=pad

    ro = row[order]
    xr_vals = x[ro]
    degr_vals = deg8[ro]
    del ro

    XR = np.zeros(NSTREAM * L, dtype=np.float32)
    DG = np.zeros(NSTREAM * L, dtype=np.uint8)
    pos = stream_s * L + slot
    XR[pos] = xr_vals
    DG[pos] = degr_vals
    XR = XR.reshape(NCORES, NR, 128, L)
    DG = DG.reshape(NCORES, NR, 128, L)
    del pos, xr_vals, degr_vals, stream_s, slot, order, stream

    # E_pos per (core, g, r, col): end slot (within partition stream) of the
    # last edge of cols <= this col in the same p16 block; 0 if none.
    CC = np.zeros((NCORES, PCOLS), dtype=np.int64)
    CC[:, :COLS_CORE] = deg.reshape(NCORES, COLS_CORE)
    CC4 = CC.reshape(NCORES, NG, NR, CG)
    cum = np.cumsum(CC4, axis=-1)
    # exclusive start of each p16 block
    blk_excl = np.zeros((NCORES, NG, NR, 16), dtype=np.int64)
    blk_excl[..., 1:] = cum[..., _BOUND[1:-1] - 1]
    endslot = cum - blk_excl[..., PMAP]                      # [8,8,32,2450]
    assert endslot.max() <= L - 1

    EPOS = np.zeros((NCORES, NG, NR, NI), dtype=np.uint16)
    EPOS[..., 1:1 + CG] = endslot.astype(np.uint16)
    # wrap (s p): epos_w[core, r, g*16+j, s] = EPOS[core, g, r, s*16+j]
    epw = EPOS.reshape(NCORES, NG, NR, NIW, 16)
    epw = np.moveaxis(epw, (1, 4), (2, 3))                   # [core, r, g, j, s]
    EPOS_W = np.ascontiguousarray(
        epw.reshape(NCORES, NR, NG * 16, NIW), dtype=np.uint16)

    # owner mask [128, NI] (same for every group/round/core)
    maskP = np.zeros((128, NI), dtype=np.float32)
    p16_of = np.arange(128) % 16
    maskP[p16_of == 0, 0] = 1.0
    owner = PMAP[None, :] == p16_of[:, None]                 # [128, 2450]
    maskP[:, 1:1 + CG] = owner.astype(np.float32)

    # PE constants
    tri = np.zeros((128, 128), dtype=np.float32)
    for gg in range(NG):
        for p in range(16):
            for q in range(p):
                tri[gg * 16 + q, gg * 16 + p] = 1.0
    wgrp = np.zeros((128, 8), dtype=np.float32)
    for gg in range(NG):
        wgrp[gg * 16:(gg + 1) * 16, gg] = 1.0

    # per-core node arrays in (g, r16, c') = natural [128, 4900] layout
    xpad = np.zeros((NCORES, PCOLS), dtype=np.float32)
    xpad[:, :COLS_CORE] = x.reshape(NCORES, COLS_CORE)
    XL = xpad.reshape(NCORES, 128, 98, 50)
    degpad = np.zeros((NCORES, PCOLS), dtype=np.uint8)
    degpad[:, :COLS_CORE] = deg8.reshape(NCORES, COLS_CORE)
    DL = degpad.reshape(NCORES, 128, 98, 50)

    fc_w = np.asarray(fc_w, dtype=np.float32)
    fcw0 = np.broadcast_to(fc_w[0], (128, 98, 50)).copy()
    fcw1 = np.broadcast_to(fc_w[1], (128, 98, 50)).copy()
    w_rep = np.full((128, 1), np.float32(np.asarray(conv_w).reshape(-1)[0]))
    b_rep = np.full((128, 1), np.float32(np.asarray(conv_b).reshape(-1)[0]))
    fb = np.asarray(fc_b, dtype=np.float32).reshape(-1)
    fcb0 = np.full((128, 1), fb[0], dtype=np.float32)
    fcb1 = np.full((128, 1), fb[1], dtype=np.float32)

    in_maps = []
    for c in range(NCORES):
        in_maps.append({
            "xr": XR[c], "degr": DG[c], "epos": EPOS_W[c],
            "maskp": maskP, "tri": tri, "wgrp": wgrp,
            "xloc": XL[c], "degloc": DL[c],
            "fcw0": fcw0, "fcw1": fcw1,
            "wrep": w_rep, "brep": b_rep, "fcb0": fcb0, "fcb1": fcb1,
        })
    return in_maps


def _build_program():
    from contextlib import ExitStack
    from concourse import bacc, tile, mybir

    nc = bacc.Bacc("TRN2", target_bir_lowering=False, debug=False,
                   num_devices=NCORES)
    f32, u8, u16 = mybir.dt.float32, mybir.dt.uint8, mybir.dt.uint16
    Alu = mybir.AluOpType
    Act = mybir.ActivationFunctionType

    d_xr = nc.dram_tensor("xr", [NR, 128, L], f32, kind="ExternalInput").ap()
    d_dg = nc.dram_tensor("degr", [NR, 128, L], u8, kind="ExternalInput").ap()
    d_ep = nc.dram_tensor("epos", [NR, 128, NIW], u16, kind="ExternalInput").ap()
    d_mk = nc.dram_tensor("maskp", [128, NI], f32, kind="ExternalInput").ap()
    d_tri = nc.dram_tensor("tri", [128, 128], f32, kind="ExternalInput").ap()
    d_wg = nc.dram_tensor("wgrp", [128, 8], f32, kind="ExternalInput").ap()
    d_xl = nc.dram_tensor("xloc", [128, 98, 50], f32, kind="ExternalInput").ap()
    d_dl = nc.dram_tensor("degloc", [128, 98, 50], u8, kind="ExternalInput").ap()
    d_f0 = nc.dram_tensor("fcw0", [128, 98, 50], f32, kind="ExternalInput").ap()
    d_f1 = nc.dram_tensor("fcw1", [128, 98, 50], f32, kind="ExternalInput").ap()
    d_wr = nc.dram_tensor("wrep", [128, 1], f32, kind="ExternalInput").ap()
    d_br = nc.dram_tensor("brep", [128, 1], f32, kind="ExternalInput").ap()
    d_b0 = nc.dram_tensor("fcb0", [128, 1], f32, kind="ExternalInput").ap()
    d_b1 = nc.dram_tensor("fcb1", [128, 1], f32, kind="ExternalInput").ap()
    o_l0 = nc.dram_tensor("l0", [128, 98], f32, kind="ExternalOutput").ap()
    o_l1 = nc.dram_tensor("l1", [128, 98], f32, kind="ExternalOutput").ap()

    # fixed-address accumulators for per-col selected cumsums
    t_Vr = nc.alloc_sbuf_tensor("t_Vr", [128, 98, 50], f32).ap()
    t_Vs = nc.alloc_sbuf_tensor("t_Vs", [128, 98, 50], f32).ap()
    t_l0 = nc.alloc_sbuf_tensor("t_l0", [128, 98], f32).ap()
    t_l1 = nc.alloc_sbuf_tensor("t_l1", [128, 98], f32).ap()

    with tile.TileContext(nc) as tc, ExitStack() as ctx:
        cpool = ctx.enter_context(tc.tile_pool(name="consts", bufs=1))
        t_mk = cpool.tile([128, NI], f32)
        nc.sync.dma_start(t_mk[:], d_mk[:])
        t_tri = cpool.tile([128, 128], f32)
        nc.sync.dma_start(t_tri[:], d_tri[:])
        t_wg = cpool.tile([128, 8], f32)
        nc.sync.dma_start(t_wg[:], d_wg[:])

        sctx = ctx.enter_context(ExitStack())
        spool = sctx.enter_context(tc.tile_pool(name="stream", bufs=2))
        wpool = sctx.enter_context(tc.tile_pool(name="scratch", bufs=1))
        wpool2 = sctx.enter_context(tc.tile_pool(name="scratch2", bufs=2))
        psum = sctx.enter_context(tc.tile_pool(name="ps", bufs=2, space="PSUM"))
        psumv = sctx.enter_context(tc.tile_pool(name="psv", bufs=1, space="PSUM"))

        for r in range(NR):
            rr, half = r // 2, r % 2
            t_xr = spool.tile([128, L], f32, tag="xr")
            nc.sync.dma_start(t_xr[:], d_xr[r])
            t_dg = spool.tile([128, L], u8, tag="dg")
            nc.sync.dma_start(t_dg[:], d_dg[r])
            t_ep = spool.tile([128, NIW], u16, tag="ep")
            nc.sync.dma_start(t_ep[:], d_ep[r])

            t_d = wpool2.tile([128, L], f32, tag="d")
            nc.scalar.activation(t_d[:], t_dg[:], Act.Abs_reciprocal_sqrt,
                                 bias=1.0, scale=1.0)
            t_v = wpool.tile([128, L], f32, tag="v")
            nc.vector.tensor_tensor(t_v[:], t_xr[:], t_d[:], Alu.mult)
            t_S = spool.tile([128, L], f32, tag="S")
            nc.vector.tensor_tensor_scan(t_S[:], t_v[:], t_v[:], 0.0,
                                         Alu.add, Alu.bypass)

            t_X = wpool2.tile([128, NI], f32, tag="X")
            for c0 in range(0, NI, 1024):
                c1 = min(NI, c0 + 1024)
                nc.gpsimd.indirect_copy(t_X[:, c0:c1], t_S[:],
                                        t_ep[:, c0 // 16:c1 // 16], True)

            p_B = psum.tile([128, 1], f32, tag="pB")
            nc.tensor.matmul(p_B[:], t_tri[:], t_S[:, L - 1:L],
                             start=True, stop=True)
            t_B = wpool.tile([128, 1], f32, tag="B")
            nc.scalar.copy(t_B[:], p_B[:])

            t_M = wpool.tile([128, NI], f32, tag="M")
            nc.vector.scalar_tensor_tensor(t_M[:], t_X[:], t_B[:, 0:1],
                                           t_mk[:], Alu.add, Alu.mult)

            t_stag = wpool2.tile([8, NI], f32, tag="stag")
            p_V = psumv.tile([8, NI], f32, tag="pV")
            for ci in range((NI + 511) // 512):
                c0, c1 = ci * 512, min(NI, ci * 512 + 512)
                nc.tensor.matmul(p_V[:, c0:c1], t_wg[:], t_M[:, c0:c1],
                                 start=True, stop=True)
            nc.scalar.copy(t_stag[:], p_V[:])

            # scatter V into final [128, 98, 50] layout:
            # dest partitions rr::16 (8 of them), free box [half*49:(+49), 0:50]
            nc.sync.dma_start(
                t_Vr[rr::16, half * 49:half * 49 + 49, :],
                t_stag[:, 1:1 + CG].rearrange("p (a b) -> p a b", b=50))
            nc.sync.dma_start(
                t_Vs[rr::16, half * 49:half * 49 + 49, :],
                t_stag[:, 0:CG].rearrange("p (a b) -> p a b", b=50))

        sctx.close()   # free stream pools before phase B

        # ---------------- phase B: per-column + FC ----------------
        bpool = ctx.enter_context(tc.tile_pool(name="phaseb", bufs=1))
        t_xl = bpool.tile([128, 98, 50], f32, tag="bufA")
        nc.sync.dma_start(t_xl[:], d_xl[:])
        t_dl = bpool.tile([128, 98, 50], u8, tag="dl")
        nc.sync.dma_start(t_dl[:], d_dl[:])
        t_wr = bpool.tile([128, 1], f32, tag="wr")
        nc.sync.dma_start(t_wr[:], d_wr[:])
        t_br = bpool.tile([128, 1], f32, tag="br")
        nc.sync.dma_start(t_br[:], d_br[:])
        t_b0 = bpool.tile([128, 1], f32, tag="b0")
        nc.sync.dma_start(t_b0[:], d_b0[:])
        t_b1 = bpool.tile([128, 1], f32, tag="b1")
        nc.sync.dma_start(t_b1[:], d_b1[:])

        t_part = bpool.tile([128, 98, 50], f32, tag="part")
        nc.vector.tensor_tensor(t_part[:], t_Vr[:], t_Vs[:], Alu.subtract)
        t_dis = bpool.tile([128, 98, 50], f32, tag="dis")
        nc.scalar.activation(t_dis[:], t_dl[:], Act.Abs_reciprocal_sqrt,
                             bias=1.0, scale=1.0)
        t_h = bpool.tile([128, 98, 50], f32, tag="h")
        nc.vector.tensor_tensor(t_h[:], t_dis[:], t_xl[:], Alu.mult)
        nc.vector.tensor_tensor(t_h[:], t_h[:], t_part[:], Alu.add)
        # h = (h * w) * dis + b
        nc.vector.scalar_tensor_tensor(t_h[:], t_h[:], t_wr[:, 0:1], t_dis[:],
                                       Alu.mult, Alu.mult)
        nc.vector.tensor_scalar_add(t_h[:], t_h[:], t_br[:, 0:1])
        # logits: q reuses bufB (sd dead), fcw tiles reuse bufA (xl dead)
        t_f0 = bpool.tile([128, 98, 50], f32, tag="part")
        nc.sync.dma_start(t_f0[:], d_f0[:])
        t_q = bpool.tile([128, 98, 50], f32, tag="bufB")
        nc.vector.tensor_tensor(t_q[:], t_h[:], t_f0[:], Alu.mult)
        nc.vector.tensor_reduce(t_l0[:], t_q[:], mybir.AxisListType.X, Alu.add)
        nc.vector.tensor_scalar_add(t_l0[:], t_l0[:], t_b0[:, 0:1])
        t_f1 = bpool.tile([128, 98, 50], f32, tag="bufA")
        nc.sync.dma_start(t_f1[:], d_f1[:])
        t_q2 = bpool.tile([128, 98, 50], f32, tag="dis")
        nc.vector.tensor_tensor(t_q2[:], t_h[:], t_f1[:], Alu.mult)
        nc.vector.tensor_reduce(t_l1[:], t_q2[:], mybir.AxisListType.X, Alu.add)
        nc.vector.tensor_scalar_add(t_l1[:], t_l1[:], t_b1[:, 0:1])

    with nc.semaphore("out_sem") as out_sem:
        nc.sync.dma_start(o_l0[:], t_l0[:]).then_inc(out_sem, 16)
        nc.sync.dma_start(o_l1[:], t_l1[:]).then_inc(out_sem, 16)
        nc.sync.wait_ge(out_sem, 32)
    nc.compile()
    return nc


def kernel(x, edge_index, conv_w, conv_b, fc_w, fc_b):
    from concourse.bass_utils import run_bass_kernel_spmd

    in_maps = _host_prep(x, edge_index, conv_w, conv_b, fc_w, fc_b)
    if "nc" not in _PROGRAM:
        _PROGRAM["nc"] = _build_program()
    nc = _PROGRAM["nc"]
    res = run_bass_kernel_spmd(nc, in_maps, list(range(NCORES)),
                               **_PROGRAM.get("run_kwargs", {}))
    out = np.zeros((NCORES, GRAPHS_CORE, 2), dtype=np.float32)
    for c in range(NCORES):
        l0 = res.results[c]["l0"].reshape(PGRAPHS)
        l1 = res.results[c]["l1"].reshape(PGRAPHS)
        out[c, :, 0] = l0[:GRAPHS_CORE]
        out[c, :, 1] = l1[:GRAPHS_CORE]
    return out.reshape(NCORES * GRAPHS_CORE, 2)

